# revision 1
# baseline (speedup 1.0000x reference)
"""Multi-head attention block on 8 Trainium2 NeuronCores.

Sharding: batch (B=2) x head-groups (4 heads each) -> 8 cores.
Each core computes q/k/v projections for its 4 heads of its batch,
causal attention in transposed (scores^T) layout, and a partial output
projection; the host sums the 4 partials per batch and adds the bias.

Layout trick: everything is kept "head-transposed" ([head_dim, seq]) so
no on-chip transposes are needed:
  qT/kT  = W @ x^T           (x^T supplied pre-transposed from host)
  S^T    = kT^T-slices @ qT  (scores block [k,q], softmax denom via a
                              ones-column appended to v in the PV matmul)
  A^T    = (V^T E)*recip     ([head_dim, q], feeds out-proj as lhsT)
Biases: b_q/b_k applied on-chip (per-partition); b_v and b_out folded
into a host-side constant (b_out + w_out @ b_v) added after gathering.
"""

import os
import re
import sys

sys.path.insert(0, "/opt/trn_rl_repo")

import numpy as np
import ml_dtypes

import concourse.bass as bass
import concourse.mybir as mybir
import concourse.tile as tile

BF16 = mybir.dt.bfloat16
F32 = mybir.dt.float32
BF16_NP = ml_dtypes.bfloat16

N_CORES = 8
B = 2
S = 2048
D_MODEL = 1024
H_TOTAL = 16
D_K = 64
H_PER_CORE = 4                      # heads per core
HD = H_PER_CORE * D_K               # 256 head-dims per core
CORES_PER_BATCH = N_CORES // B

QB = 512                            # q-block (matmul moving free dim)
KC = 128                            # k-chunk (contraction tile)

N_PROCS = 27


def _split_waits_json(bir_json: bytes, limit: int = 1) -> bytes:
    """walrus in this env rejects >limit sync-waits on an instruction.
    Hoist excess waits onto fresh NoOps inserted just before, on the same
    engine queue (queue execution is serial, so ordering is identical)."""
    import orjson

    m = orjson.loads(bir_json)
    ctr = 0
    for fn in m.get("functions", []):
        for bb in fn.get("blocks", []):
            insts = bb.get("instructions") or []
            if not any(
                len((i.get("sync_info") or {}).get("on_wait") or []) > limit
                for i in insts
            ):
                continue
            out = []
            for inst in insts:
                si = inst.get("sync_info")
                waits = (si or {}).get("on_wait") or []
                if len(waits) > limit:
                    for w in waits[:-limit]:
                        ctr += 1
                        out.append(
                            {
                                "debug": inst.get("debug", 0),
                                "engine": inst["engine"],
                                "ins": [],
                                "outs": [],
                                "name": f"WSPL-{ctr}",
                                "opcode": "NoOp",
                                "sync_info": {"on_update": [], "on_wait": [w]},
                            }
                        )
                    si["on_wait"] = waits[-limit:]
                out.append(inst)
            bb["instructions"] = out
    return orjson.dumps(m)


LAST_PREDICTED_NS = None


def _install_schedule_capture():
    """Record the Tile scheduler's cost-model makespan for each build."""
    if getattr(tile.TileContext, "_capture_installed", False):
        return
    orig = tile.TileContext.schedule_block

    def wrapped(self, *a, **kw):
        r = orig(self, *a, **kw)
        try:
            global LAST_PREDICTED_NS
            LAST_PREDICTED_NS = r[1].time
        except Exception:
            pass
        return r

    tile.TileContext.schedule_block = wrapped
    tile.TileContext._capture_installed = True


def _install_compile_patch():
    import concourse.bass_utils as bu
    import concourse.bass2jax as b2j

    if getattr(bu, "_wait_split_installed", False):
        return
    orig = bu.compile_bir_kernel

    def wrapped(bir_json, tmpdir, neff_name="file.neff"):
        return orig(_split_waits_json(bytes(bir_json)), tmpdir, neff_name)

    bu.compile_bir_kernel = wrapped
    b2j.compile_bir_kernel = wrapped
    bu._wait_split_installed = True


def build_program(mask_mode="causal", s=S, d=D_MODEL, heads=H_PER_CORE,
                  epool_bufs=8, apool_bufs=3, opool_bufs=4):
    """One SPMD program; per-core behavior differs only via inputs.

    mask_mode: "causal" (skip above-diagonal chunks, affine-select the
    diagonal ones), "ones" (no masking), "general" (multiplicative 0/1
    mask loaded from DRAM, pre-transposed host-side).
    """
    _install_compile_patch()
    _install_schedule_capture()
    hd = heads * D_K
    nq = s // QB          # q blocks
    nkc = s // KC         # k chunks
    dch = d // 128        # contraction chunks for projections
    npair = heads // 2    # head pairs (even head on partitions 0-63)
    assert hd % 128 == 0 and hd // 128 == npair

    nc = bass.Bass()
    xq = nc.dram_tensor("xq", [d, s], BF16, kind="ExternalInput")
    xk = nc.dram_tensor("xk", [d, s], BF16, kind="ExternalInput")
    xv = nc.dram_tensor("xv", [d, s], BF16, kind="ExternalInput")
    # weights arrive pre-packed host-side so each partition's line is one
    # contiguous 4KB run: wq[p, c*hd+m] = w_q.T[c*128+p, m] etc.
    wq = nc.dram_tensor("wq", [128, dch * hd], BF16, kind="ExternalInput")
    wk = nc.dram_tensor("wk", [128, dch * hd], BF16, kind="ExternalInput")
    wv = nc.dram_tensor("wv", [128, dch * hd], BF16, kind="ExternalInput")
    wo = nc.dram_tensor("wo", [128, npair * d], BF16, kind="ExternalInput")
    bq = nc.dram_tensor("bq", [hd, 1], F32, kind="ExternalInput")
    bk = nc.dram_tensor("bk", [hd, 1], F32, kind="ExternalInput")
    if mask_mode == "general":
        m01 = nc.dram_tensor("m01", [s, s], BF16, kind="ExternalInput")
    out = nc.dram_tensor("out", [s, d], BF16, kind="ExternalOutput")

    xq_r = xq[:, :].rearrange("(c p) s -> p c s", p=128)
    xk_r = xk[:, :].rearrange("(c p) s -> p c s", p=128)
    xv_r = xv[:, :].rearrange("(c p) s -> p c s", p=128)
    wq_r = wq[:, :].rearrange("p (c m) -> p c m", m=hd)
    wk_r = wk[:, :].rearrange("p (c m) -> p c m", m=hd)
    wv_r = wv[:, :].rearrange("p (c m) -> p c m", m=hd)
    wo_r = wo[:, :].rearrange("p (c e) -> p c e", e=d)

    with tile.TileContext(nc) as tc:
        with (
            tc.tile_pool(name="consts", bufs=1) as consts,
            tc.tile_pool(name="qkres", bufs=1) as qkres,
            tc.tile_pool(name="epool", bufs=epool_bufs) as epool,
            tc.tile_pool(name="apool", bufs=apool_bufs) as apool,
            tc.tile_pool(name="opool", bufs=opool_bufs) as opool,
            tc.tile_pool(name="rpool", bufs=4) as rpool,
            tc.tile_pool(name="mpool", bufs=16) as mpool,
            tc.tile_pool(name="pp", bufs=2, space="PSUM") as pp,
            tc.tile_pool(name="sp", bufs=2, space="PSUM") as sp,
            tc.tile_pool(name="app", bufs=2, space="PSUM") as app,
        ):
            # --- constants + resident x inputs, ordered by first use.
            # x loads are split into s-halves per D-chunk: 2KB partition
            # lines (good DMA shape) and per-half tiles keep dependency
            # granularity fine so qb0's projections start early.
            wk_sb = consts.tile([128, dch, hd], BF16, tag="wk")
            nc.sync.dma_start(wk_sb, wk_r)
            bk_sb = consts.tile([128, npair], F32, tag="bk")
            bq_sb = consts.tile([128, npair], F32, tag="bq")
            for c2 in range(npair):
                nc.sync.dma_start(bk_sb[:, c2 : c2 + 1], bk[c2 * 128 : (c2 + 1) * 128, :])
                nc.sync.dma_start(bq_sb[:, c2 : c2 + 1], bq[c2 * 128 : (c2 + 1) * 128, :])

            ones64 = consts.tile([1, 64], BF16, tag="ones")
            nc.vector.memset(ones64, 1.0)

            sh = s // 2
            xk_h = [qkres.tile([128, dch, sh], BF16, tag="xk", name=f"xk{i}") for i in range(2)]
            xv_h = [qkres.tile([128, dch, sh], BF16, tag="xv", name=f"xv{i}") for i in range(2)]
            xq_h = [qkres.tile([128, dch, sh], BF16, tag="xq", name=f"xq{i}") for i in range(2)]
            wq_sb = consts.tile([128, dch, hd], BF16, tag="wq")
            nc.sync.dma_start(wq_sb, wq_r)
            for dc in range(dch):
                nc.sync.dma_start(xk_h[0][:, dc, :], xk_r[:, dc, 0:sh])
                nc.sync.dma_start(xq_h[0][:, dc, :], xq_r[:, dc, 0:sh])
            wv_sb = consts.tile([128, dch, hd], BF16, tag="wv")
            nc.sync.dma_start(wv_sb, wv_r)
            for dc in range(dch):
                nc.sync.dma_start(xv_h[0][:, dc, :], xv_r[:, dc, 0:sh])
            wo_sb = consts.tile([128, npair, d], BF16, tag="wo")
            nc.sync.dma_start(wo_sb, wo_r)
            for dc in range(dch):
                nc.sync.dma_start(xk_h[1][:, dc, :], xk_r[:, dc, sh:s])
            for dc in range(dch):
                nc.sync.dma_start(xq_h[1][:, dc, :], xq_r[:, dc, sh:s])
            for dc in range(dch):
                nc.sync.dma_start(xv_h[1][:, dc, :], xv_r[:, dc, sh:s])

            # persistent per-core tensors
            qT = qkres.tile([128, npair, s], BF16, tag="qT")
            kT = qkres.tile([128, npair, s], BF16, tag="kT")
            v_sb = qkres.tile([128, nkc, heads * 65], BF16, tag="v")
            # fill with 1.0 once; v-proj copies overwrite cols 0:64 of each
            # 65-block, leaving column 64 = 1.0 (softmax denominator trick)
            nc.vector.memset(v_sb, 1.0)

            def proj_block(qb):
                s_lo = qb * QB
                half = (qb * QB) // sh
                h_lo = s_lo - half * sh
                xk_t, xq_t, xv_t = xk_h[half], xq_h[half], xv_h[half]

                def kq_proj_group(x_t, w_sb, b_sb, dst, c2):
                    ps = pp.tile([128, QB], F32, tag="pp", name="ps")
                    for dc in range(dch):
                        nc.tensor.matmul(
                            ps,
                            lhsT=w_sb[:, dc, c2 * 128 : (c2 + 1) * 128],
                            rhs=x_t[:, dc, h_lo : h_lo + QB],
                            start=(dc == 0),
                            stop=(dc == dch - 1),
                        )
                    nc.vector.tensor_scalar_add(
                        dst[:, c2, s_lo : s_lo + QB], ps, b_sb[:, c2 : c2 + 1]
                    )

                def v_proj_group(sc):
                    sck = qb * (QB // 128) + sc
                    ps = pp.tile([128, hd], F32, tag="pp", name="ps")
                    for dc in range(dch):
                        nc.tensor.matmul(
                            ps,
                            lhsT=xv_t[:, dc, h_lo + sc * 128 : h_lo + (sc + 1) * 128],
                            rhs=wv_sb[:, dc, :],
                            start=(dc == 0),
                            stop=(dc == dch - 1),
                        )
                    nc.vector.tensor_copy(
                        v_sb[:, sck, :].rearrange("p (h j) -> p h j", j=65)[:, :, 0:64],
                        ps[:].rearrange("p (h j) -> p h j", j=64),
                    )

                for c2 in range(npair):
                    kq_proj_group(xk_t, wk_sb, bk_sb, kT, c2)
                for c2 in range(npair):
                    kq_proj_group(xq_t, wq_sb, bq_sb, qT, c2)
                for sc in range(QB // 128):
                    v_proj_group(sc)

            def attn_block(qb):
                s_lo = qb * QB
                # --- attention: k-chunk PAIRS share one [128, 2*QB] psum
                # tile (2 banks) so exp runs as a single wide ACT op ---
                n_chunks = (qb + 1) * (QB // KC) if mask_mode == "causal" else nkc
                diag_lo = qb * (QB // KC)
                a_sb = apool.tile([128, npair, QB], BF16, tag="a")
                if mask_mode == "general":
                    m_tiles = []
                    for kc_i in range(n_chunks):
                        mt = mpool.tile([128, QB], BF16, tag="m")
                        nc.sync.dma_start(
                            mt, m01[kc_i * KC : (kc_i + 1) * KC, s_lo : s_lo + QB]
                        )
                        m_tiles.append(mt)
                assert n_chunks % 2 == 0
                for pr in range(npair):
                    a_ps = [app.tile([65, QB], F32, tag="app", name=f"aps{sub_i}") for sub_i in range(2)]
                    for kcp in range(0, n_chunks, 2):
                        # fully-masked column prefix per chunk (diagonal band)
                        if mask_mode == "causal":
                            skips = [
                                max(0, ((kcp + ck) - diag_lo) * KC)
                                if (kcp + ck) >= diag_lo
                                else 0
                                for ck in range(2)
                            ]
                        else:
                            skips = [0, 0]
                        e_t = [None, None]
                        s_pss = [None, None]
                        # emit the four score matmuls alternating head-subs:
                        # adjacent MMs then occupy disjoint PE row-groups
                        # (partitions 0-63 vs 64-127) and overlap in HW
                        for sub in range(2):
                            s_pss[sub] = sp.tile([128, 2 * QB], F32, tag="sp", name=f"sps{sub}")
                        for ck in range(2):
                            sk = skips[ck]
                            for sub in range(2):
                                rows = slice(sub * 64, sub * 64 + 64)
                                nc.tensor.matmul(
                                    s_pss[sub][:, ck * QB + sk : (ck + 1) * QB],
                                    lhsT=kT[rows, pr, (kcp + ck) * KC : (kcp + ck + 1) * KC],
                                    rhs=qT[rows, pr, s_lo + sk : s_lo + QB],
                                    start=True,
                                    stop=True,
                                )
                        for sub in range(2):
                            s_ps = s_pss[sub]
                            e = epool.tile([128, 2 * QB], BF16, tag="e")
                            if skips[0] == 0 and skips[1] == 0:
                                nc.scalar.activation(
                                    out=e, in_=s_ps,
                                    func=mybir.ActivationFunctionType.Exp,
                                )
                            else:
                                for ck in range(2):
                                    sk = skips[ck]
                                    nc.scalar.activation(
                                        out=e[:, ck * QB + sk : (ck + 1) * QB],
                                        in_=s_ps[:, ck * QB + sk : (ck + 1) * QB],
                                        func=mybir.ActivationFunctionType.Exp,
                                    )
                            for ck in range(2):
                                kc_i = kcp + ck
                                sk = skips[ck]
                                if mask_mode == "causal" and kc_i >= diag_lo:
                                    # after narrowing, keep condition is
                                    # (f' - p) >= 0 with zero base
                                    nc.gpsimd.affine_select(
                                        out=e[:, ck * QB + sk : (ck + 1) * QB],
                                        in_=e[:, ck * QB + sk : (ck + 1) * QB],
                                        compare_op=mybir.AluOpType.is_ge,
                                        fill=0.0,
                                        base=0,
                                        pattern=[[1, QB - sk]],
                                        channel_multiplier=-1,
                                    )
                                if mask_mode == "general":
                                    nc.vector.tensor_mul(
                                        e[:, ck * QB : (ck + 1) * QB],
                                        e[:, ck * QB : (ck + 1) * QB],
                                        m_tiles[kc_i],
                                    )
                            e_t[sub] = e
                        for sub in range(2):
                            h = pr * 2 + sub
                            for ck in range(2):
                                kc_i = kcp + ck
                                sk = skips[ck]
                                nc.tensor.matmul(
                                    a_ps[sub][:, sk:],
                                    lhsT=v_sb[:, kc_i, h * 65 : (h + 1) * 65],
                                    rhs=e_t[sub][:, ck * QB + sk : (ck + 1) * QB],
                                    start=(kc_i == 0),
                                    stop=(kc_i == n_chunks - 1),
                                )
                    for sub in range(2):
                        rows = slice(sub * 64, sub * 64 + 64)
                        recip = rpool.tile([1, QB], BF16, tag="r")
                        with nc.allow_low_precision(reason="bf16 recip, gate is 2e-2"):
                            nc.vector.reciprocal(out=recip, in_=a_ps[sub][64:65, :])
                        bpool, btag = (pp, "pp") if pr % 2 == 0 else (sp, "sp")
                        b_ps = bpool.tile([64, QB], F32, tag=btag, name="bps")
                        nc.tensor.matmul(
                            b_ps, lhsT=ones64, rhs=recip, start=True, stop=True
                        )
                        rb = rpool.tile([64, QB], F32, tag="rb")
                        nc.vector.tensor_copy(rb, b_ps)
                        nc.vector.tensor_mul(
                            a_sb[rows, pr, :], a_ps[sub][0:64, :], rb
                        )

                # --- output projection for this q block ---
                for qc in range(QB // 128):
                    o_sb = opool.tile([128, d], BF16, tag="o")
                    for nb in range(d // QB):
                        # rotate over sp/app (both free at the qb tail) so
                        # pp stays available for the next block's projections
                        pool, ptag = (sp, "sp") if nb % 2 == 0 else (app, "app")
                        o_ps = pool.tile([128, QB], F32, tag=ptag, name="ops")
                        for c2 in range(npair):
                            nc.tensor.matmul(
                                o_ps,
                                lhsT=a_sb[:, c2, qc * 128 : (qc + 1) * 128],
                                rhs=wo_sb[:, c2, nb * QB : (nb + 1) * QB],
                                start=(c2 == 0),
                                stop=(c2 == npair - 1),
                            )
                        if nb % 2 == 0:
                            nc.scalar.copy(o_sb[:, nb * QB : (nb + 1) * QB], o_ps)
                        else:
                            nc.vector.tensor_copy(o_sb[:, nb * QB : (nb + 1) * QB], o_ps)
                    nc.sync.dma_start(
                        out[(s_lo + qc * 128) : (s_lo + (qc + 1) * 128), :], o_sb
                    )

            if mask_mode == "causal":
                # attention(qb) only reads k/v ranges projected so far
                for qb in range(nq):
                    proj_block(qb)
                    attn_block(qb)
            else:
                # unmasked attention reads ALL k/v: project everything first
                for qb in range(nq):
                    proj_block(qb)
                for qb in range(nq):
                    attn_block(qb)

    return nc


# ---------------------------------------------------------------------------
# host side
# ---------------------------------------------------------------------------

_PROG_CACHE = {}
LAST_RESULT = None


def _get_program(mask_mode):
    if mask_mode not in _PROG_CACHE:
        _PROG_CACHE[mask_mode] = build_program(mask_mode)
    return _PROG_CACHE[mask_mode]


def _bf16(a):
    return np.ascontiguousarray(a).astype(BF16_NP)


def _pack_w(wT):
    """[D, m] -> [128, (D//128)*m] with row p holding chunks contiguously."""
    dch_, m = wT.shape[0] // 128, wT.shape[1]
    return np.ascontiguousarray(
        wT.reshape(dch_, 128, m).transpose(1, 0, 2).reshape(128, dch_ * m)
    )


def kernel(query, key_in, value, mask, w_q, b_q, w_k, b_k, w_v, b_v, w_out, b_out):
    from concourse.bass_utils import run_bass_kernel_spmd

    query = np.asarray(query, dtype=np.float32)
    key_in = np.asarray(key_in, dtype=np.float32)
    value = np.asarray(value, dtype=np.float32)
    mask = np.asarray(mask)
    w_q = np.asarray(w_q, dtype=np.float32)
    b_q = np.asarray(b_q, dtype=np.float32)
    w_k = np.asarray(w_k, dtype=np.float32)
    b_k = np.asarray(b_k, dtype=np.float32)
    w_v = np.asarray(w_v, dtype=np.float32)
    b_v = np.asarray(b_v, dtype=np.float32)
    w_out = np.asarray(w_out, dtype=np.float32)
    b_out = np.asarray(b_out, dtype=np.float32)

    scale = 1.0 / np.sqrt(np.float32(D_K))

    if (mask == 1).all():
        mode = "ones"
    elif all(
        np.array_equal(mask[b, 0], np.tril(np.ones((S, S), mask.dtype)))
        for b in range(mask.shape[0])
    ):
        mode = "causal"
    else:
        mode = "general"
    nc = _get_program(mode)

    wqT = _bf16(w_q.T * scale)   # [D, D] scaled
    wkT = _bf16(w_k.T)
    wvT = _bf16(w_v.T)
    woT = _bf16(w_out.T)
    bq_s = (b_q * scale).astype(np.float32)

    # per-batch transposed activations, shared by the 4 cores of a batch
    xqT = [_bf16(query[b].T) for b in range(B)]
    xkT = [_bf16(key_in[b].T) for b in range(B)]
    xvT = [_bf16(value[b].T) for b in range(B)]
    m01T = [_bf16(mask[b, 0].T) for b in range(B)] if mode == "general" else None

    in_maps = []
    for c in range(N_CORES):
        b = c // CORES_PER_BATCH
        hg = c % CORES_PER_BATCH
        hsl = slice(hg * HD, (hg + 1) * HD)
        im = {
            "xq": xqT[b],
            "xk": xkT[b],
            "xv": xvT[b],
            "wq": _pack_w(wqT[:, hsl]),
            "wk": _pack_w(wkT[:, hsl]),
            "wv": _pack_w(wvT[:, hsl]),
            "wo": _pack_w(woT[hsl, :]),
            "bq": np.ascontiguousarray(bq_s[hsl].reshape(HD, 1)),
            "bk": np.ascontiguousarray(b_k[hsl].reshape(HD, 1)),
        }
        if mode == "general":
            im["m01"] = m01T[b]
        in_maps.append(im)

    global LAST_RESULT
    try:
        res = run_bass_kernel_spmd(nc, in_maps, list(range(N_CORES)))
    except Exception:
        # transient NRT_EXEC_UNIT_UNRECOVERABLE wedges have been observed on
        # this fabric; a single retry has always cleared them
        import time as _time

        _time.sleep(3.0)
        res = run_bass_kernel_spmd(nc, in_maps, list(range(N_CORES)))
    LAST_RESULT = res

    b_eff = b_out + w_out @ b_v
    out = np.zeros((B, S, D_MODEL), dtype=np.float32)
    for c in range(N_CORES):
        out[c // CORES_PER_BATCH] += res.results[c]["out"].astype(np.float32)
    out += b_eff[None, None, :]
    return out

